# revision 33
# baseline (speedup 1.0000x reference)
"""Trainium2 Bass kernel for nn_DiagRNN (diagonal complex linear RNN / LRU).

  y = Re[C @ h] + D*x,  h_t = A h_{t-1} + B x_t  (A complex-diagonal)

Strategy (8 NeuronCores, sequence-parallel), v5:
  * L=16384 split into 32 chunks of T=512; chunk m on core m%8, slot m//8.
    Slots processed sequentially (4 phases) so the per-slot AllGather
    barriers keep cores loosely synced; consume work for slot s is emitted
    behind slot s+2's B matmuls (engine queues are in-order).
  * Complex scan -> two real scans per chunk via rotating-frame transform.
    Elementwise rotation works on packed [re|im] tiles [128,1024]:
        P  = [p_re | p_im]             (Act copies from PSUM)
        CP = [cos|cos] * P             (DVE, broadcast-pair view)
        SP = [-sin|+sin] * P           (DVE)
        g  = [CP_L + SP_R | CP_R + SP_L] = [g_re | g_im]  (2 DVE adds)
        W  = scan(r, g) per half       (DVE hw scan, zero-init, fp32 r)
        CW = [cos|-sin] * W            (DVE)
        u1 = CW_L + CW_R               (DVE)  -- carry-free output
    One [H, 3T] table [cos | -sin | +sin] serves CP/SP/CW.
  * Carries: chunk sums E (fp16) AllGather'd per slot; predecessor folds
    via small DVE ops; carry enters u via fused affine_then_add with
    tables Pc=cos*rpow, Ps=-sin*rpow.
  * y = C@u + diag(D)@x fused into the PSUM accumulation of the C matmul.
  * An early dummy AllGather absorbs cross-core launch skew.
"""
import sys, os
sys.path.insert(0, '/opt/trn_rl_repo')
import numpy as np

import concourse.bass as bass
import concourse.bacc as bacc
import concourse.tile as tile
import concourse.mybir as mybir
from concourse.bass_utils import run_bass_kernel_spmd

L, H, M = 16384, 1024, 1024
NC = 8
T = 512
S = L // (T * NC)          # 4 slots
NSL = H // 128             # 8 slices

f32 = mybir.dt.float32
f16 = mybir.dt.float16
AL = mybir.AluOpType
AX = mybir.AxisListType

_BUILD_CACHE = {}


def _build():
    if "nc" in _BUILD_CACHE:
        return _BUILD_CACHE["nc"]
    nc = bacc.Bacc("TRN2", target_bir_lowering=False, debug=False,
                   num_devices=NC)

    xt_d = nc.dram_tensor("xt", [S, M, T], f16, kind="ExternalInput").ap()
    brt_d = nc.dram_tensor("brt", [M, H], f16, kind="ExternalInput").ap()
    bit_d = nc.dram_tensor("bit", [M, H], f16, kind="ExternalInput").ap()
    ct_d = nc.dram_tensor("ct", [H, M], f16, kind="ExternalInput").ap()
    trig_d = nc.dram_tensor("trig", [H, 3 * T], f16, kind="ExternalInput").ap()
    pcs_d = nc.dram_tensor("pcs", [H, 2 * T], f16, kind="ExternalInput").ap()
    cT_d = nc.dram_tensor("cT", [128, 128], f32, kind="ExternalInput").ap()
    cwfre_d = nc.dram_tensor("cwfre", [128, 128], f32, kind="ExternalInput").ap()
    cwfim_d = nc.dram_tensor("cwfim", [128, 128], f32, kind="ExternalInput").ap()
    rwfre_d = nc.dram_tensor("rwfre", [128, 128], f32, kind="ExternalInput").ap()
    rwfim_d = nc.dram_tensor("rwfim", [128, 128], f32, kind="ExternalInput").ap()
    ident_d = nc.dram_tensor("ident", [128, 128], f16, kind="ExternalInput").ap()
    ddg_d = nc.dram_tensor("ddg", [128, M], f16, kind="ExternalInput").ap()
    y_d = nc.dram_tensor("y", [S, M, T], f16, kind="ExternalOutput").ap()

    with tile.TileContext(nc) as tc:
        with tc.tile_pool(name="pw", bufs=1) as pw, \
             tc.tile_pool(name="px", bufs=1) as px, \
             tc.tile_pool(name="pg", bufs=1) as pg, \
             tc.tile_pool(name="pc", bufs=1) as pcp, \
             tc.tile_pool(name="pp", bufs=1, space="PSUM") as pp, \
             tc.tile_pool(name="pd", bufs=1, space="DRAM") as pd:

            # ---------- persistent weights / tables ----------
            brt_sb = []
            bit_sb = []
            ct_sb = []
            trig_sb = []
            pcs_sb = []
            for d in range(NSL):
                brt_sb.append(pw.tile([128, H], f16, name=f"brt{d}"))
                bit_sb.append(pw.tile([128, H], f16, name=f"bit{d}"))
                ct_sb.append(pw.tile([128, M], f16, name=f"ct{d}"))
                trig_sb.append(pw.tile([128, 3 * T], f16, name=f"trig{d}"))
                pcs_sb.append(pw.tile([128, 2 * T], f16, name=f"pcs{d}"))

            ident_sb = pw.tile([128, 128], f16, name="ident")
            nc.sync.dma_start(ident_sb[:], ident_d)
            ddg_sb = pw.tile([128, M], f16, name="ddg")
            nc.sync.dma_start(ddg_sb[:], ddg_d)
            cT = pw.tile([128, 128], f32, name="cT")
            nc.sync.dma_start(cT[:], cT_d)
            cwfre_sb = pw.tile([128, 128], f32, name="cwfre")
            cwfim_sb = pw.tile([128, 128], f32, name="cwfim")
            rwfre_sb = pw.tile([128, 128], f32, name="rwfre")
            rwfim_sb = pw.tile([128, 128], f32, name="rwfim")

            # early sync: tiny AllGather to absorb cross-core launch skew
            dum_dr = pd.tile([16, 128], f16, name="dumdr")
            nc.sync.dma_start(dum_dr[:], ident_sb[0:16, :])
            # PE warmup during the initial DMA window: sustained matmul
            # activity trips the HAM clock gate to full rate before the
            # real B matmuls arrive.
            wm = pp.tile([128, T], f32, name="warm", tag="ytile", bufs=3)
            for i in range(40):
                nc.tensor.matmul(wm[:, 0:128], ident_sb[:], ident_sb[:],
                                 start=(i == 0), stop=(i == 39))
            dumg_dr = pd.tile([128, 128], f16, name="dumg", addr_space="Shared")
            nc.gpsimd.collective_compute(
                "AllGather", AL.bypass,
                replica_groups=[list(range(NC))],
                ins=[dum_dr[:].opt()],
                outs=[dumg_dr[:].opt()],
            )

            def emit_deferred_tables():
                for d in range(2):
                    nc.sync.dma_start(trig_sb[d][:], trig_d[d * 128:(d + 1) * 128, :])
                for d in range(NSL):
                    nc.sync.dma_start(bit_sb[d][:], bit_d[d * 128:(d + 1) * 128, :])
                for d in range(2, NSL):
                    nc.sync.dma_start(trig_sb[d][:], trig_d[d * 128:(d + 1) * 128, :])
                for d in range(NSL):
                    nc.sync.dma_start(ct_sb[d][:], ct_d[d * 128:(d + 1) * 128, :])
                    nc.sync.dma_start(pcs_sb[d][:], pcs_d[d * 128:(d + 1) * 128, :])
                nc.sync.dma_start(cwfre_sb[:], cwfre_d)
                nc.sync.dma_start(cwfim_sb[:], cwfim_d)
                nc.sync.dma_start(rwfre_sb[:], rwfre_d)
                nc.sync.dma_start(rwfim_sb[:], rwfim_d)

            def ccv(q, pt):
                # [128, 8] strided view of const block q, part pt (0=re,1=im)
                return cT[:, 16 * q + pt:16 * q + 16:2]

            def ccol(q, sl, pt):
                return cT[:, 16 * q + 2 * sl + pt:16 * q + 2 * sl + pt + 1]

            # persistent carry state
            zR_re = pcp.tile([128, 8], f32, name="zR_re")
            zR_im = pcp.tile([128, 8], f32, name="zR_im")
            nc.vector.memzero(zR_re[:])
            nc.vector.memzero(zR_im[:])
            state = {"R_re": zR_re, "R_im": zR_im}
            saved = {}
            xt_tiles = {}

            def emit_xt(s):
                xt_sb = []
                for d in range(NSL):
                    t_ = px.tile([128, T], f16, name=f"xt_s{s}_d{d}",
                                 tag="xt", bufs=24)
                    nc.sync.dma_start(t_[:], xt_d[s, d * 128:(d + 1) * 128, :])
                    xt_sb.append(t_)
                    if s == 0:
                        nc.sync.dma_start(brt_sb[d][:],
                                          brt_d[d * 128:(d + 1) * 128, :])
                xt_tiles[s] = xt_sb

            def emit_slot(s, after_xt=None):
                """B matmuls + rotate + scans + E publish for slot s."""
                xt_sb = xt_tiles[s]
                if after_xt is not None:
                    after_xt()

                u1_t = []
                wlre = pcp.tile([128, 8], f16, name=f"wlre{s}", tag="wl", bufs=2)
                wlim = pcp.tile([128, 8], f16, name=f"wlim{s}", tag="wl2", bufs=2)

                for sl in range(NSL):
                    hs = slice(sl * 128, (sl + 1) * 128)
                    ps_re = pp.tile([128, T], f32, name=f"psre{s}_{sl}",
                                    tag="bu", bufs=4)
                    ps_im = pp.tile([128, T], f32, name=f"psim{s}_{sl}",
                                    tag="bu", bufs=4)
                    for d in range(NSL):
                        nc.tensor.matmul(ps_re[:], brt_sb[d][:, hs], xt_sb[d][:],
                                         start=(d == 0), stop=(d == NSL - 1))
                    for d in range(NSL):
                        nc.tensor.matmul(ps_im[:], bit_sb[d][:, hs], xt_sb[d][:],
                                         start=(d == 0), stop=(d == NSL - 1))

                    P = pg.tile([128, 2 * T], f16, name=f"P{s}_{sl}",
                                tag="P", bufs=4)
                    nc.scalar.copy(P[:, 0:T], ps_re[:])
                    nc.scalar.copy(P[:, T:2 * T], ps_im[:])
                    CP = pg.tile([128, 2 * T], f16, name=f"cp{s}_{sl}",
                                 tag="cp", bufs=3)
                    cosD = trig_sb[sl][:, 0:T].unsqueeze(1).broadcast_to(
                        [128, 2, T])
                    nc.vector.tensor_tensor(
                        CP[:].rearrange("a (b c) -> a b c", b=2),
                        cosD, P[:].rearrange("a (b c) -> a b c", b=2),
                        AL.mult)
                    SP = pg.tile([128, 2 * T], f16, name=f"sp{s}_{sl}",
                                 tag="sp", bufs=3)
                    nc.vector.tensor_tensor(SP[:], trig_sb[sl][:, T:3 * T],
                                            P[:], AL.mult)
                    g = pg.tile([128, 2 * T], f16, name=f"g{s}_{sl}",
                                tag="g", bufs=3)
                    nc.vector.tensor_tensor(g[:, 0:T], CP[:, 0:T],
                                            SP[:, T:2 * T], AL.add)
                    nc.vector.tensor_tensor(g[:, T:2 * T], CP[:, T:2 * T],
                                            SP[:, 0:T], AL.add)
                    W = pg.tile([128, 2 * T], f16, name=f"w{s}_{sl}",
                                tag="w", bufs=4)
                    rdec = ccol(5, sl, 1).broadcast_to([128, T])
                    nc.vector.tensor_tensor_scan(W[:, 0:T], rdec, g[:, 0:T],
                                                 0.0, AL.mult, AL.add)
                    nc.vector.tensor_tensor_scan(W[:, T:2 * T], rdec,
                                                 g[:, T:2 * T],
                                                 0.0, AL.mult, AL.add)
                    CW = pg.tile([128, 2 * T], f16, name=f"cw{s}_{sl}",
                                 tag="cw", bufs=3)
                    nc.vector.tensor_tensor(CW[:], trig_sb[sl][:, 0:2 * T],
                                            W[:], AL.mult)
                    u1 = pg.tile([128, T], f16, name=f"u1_{s}_{sl}",
                                 tag="u1", bufs=26)
                    nc.vector.tensor_tensor(u1[:], CW[:, 0:T],
                                            CW[:, T:2 * T], AL.add)
                    u1_t.append(u1)
                    nc.vector.tensor_copy(wlre[:, sl:sl + 1], W[:, T - 1:T])
                    nc.vector.tensor_copy(wlim[:, sl:sl + 1], W[:, 2 * T - 1:2 * T])

                saved[s] = dict(xt_sb=xt_sb, u1_t=u1_t, wlre=wlre, wlim=wlim)

            def emit_publish(s):
                sv = saved[s]
                wlre, wlim = sv["wlre"], sv["wlim"]
                # E publish: E = ROTT1 * W_last, fp16
                epack = pcp.tile([128, 16], f16, name=f"epack{s}", tag="ep",
                                 bufs=2)
                sa = pcp.tile([128, 8], f32, name=f"sa{s}", tag="sa", bufs=2)
                sb_ = pcp.tile([128, 8], f32, name=f"sb{s}", tag="sb", bufs=2)
                sc_ = pcp.tile([128, 8], f32, name=f"sc{s}", tag="sc", bufs=2)
                sd = pcp.tile([128, 8], f32, name=f"sd{s}", tag="sd", bufs=2)
                nc.vector.tensor_tensor(sa[:], ccv(2, 0), wlre[:], AL.mult)
                nc.vector.tensor_tensor(sb_[:], ccv(2, 1), wlim[:], AL.mult)
                nc.vector.tensor_tensor(epack[:, 0:16:2], sa[:], sb_[:],
                                        AL.subtract)
                nc.vector.tensor_tensor(sc_[:], ccv(2, 0), wlim[:], AL.mult)
                nc.vector.tensor_tensor(sd[:], ccv(2, 1), wlre[:], AL.mult)
                nc.vector.tensor_tensor(epack[:, 1:16:2], sc_[:], sd[:],
                                        AL.add)

                pub_ps = pp.tile([16, 128], f16, name=f"pubps{s}", tag="tp",
                                 bufs=1)
                nc.tensor.transpose(pub_ps[:], epack[:], ident_sb[:])
                pub_sb = pcp.tile([16, 128], f16, name=f"pubsb{s}",
                                  tag="pub", bufs=2)
                nc.vector.tensor_copy(pub_sb[:], pub_ps[:])
                pub_dr = pd.tile([16, 128], f16, name=f"pubdr{s}",
                                 tag="pubd", bufs=2)
                nc.sync.dma_start(pub_dr[:], pub_sb[:])
                gat_dr = pd.tile([128, 128], f16, name=f"gatdr{s}",
                                 tag="gatd", bufs=2, addr_space="Shared")
                nc.gpsimd.collective_compute(
                    "AllGather", AL.bypass,
                    replica_groups=[list(range(NC))],
                    ins=[pub_dr[:].opt()],
                    outs=[gat_dr[:].opt()],
                )
                sv["gat_dr"] = gat_dr

            def emit_consume_carry(s):
                sv = saved[s]
                u1_t = sv["u1_t"]
                eg = pcp.tile([128, 128], f16, name=f"eg{s}", tag="eg", bufs=2)
                nc.sync.dma_start(eg[:], sv["gat_dr"][:])
                et = pcp.tile([128, 128], f16, name=f"et{s}", tag="et", bufs=2)
                nc.sync.dma_start_transpose(et[:], eg[:])

                def wsum(fold_sb, nmv, nmr1, nm):
                    tmp = pcp.tile([128, 128], f32, name=f"{nmv}{s}", tag="redt",
                                   bufs=2)
                    nc.vector.tensor_tensor(tmp[:], fold_sb[:], et[:], AL.mult)
                    red1 = pcp.tile([128, 16], f32, name=f"{nmr1}{s}",
                                    tag="red1", bufs=2)
                    nc.vector.tensor_reduce(
                        red1[:].unsqueeze(2),
                        tmp[:].rearrange("p (j x) -> p x j", j=8),
                        AX.X, AL.add)
                    out = pcp.tile([128, 8], f32, name=f"{nm}{s}", tag=nm,
                                   bufs=2)
                    nc.vector.tensor_reduce(
                        out[:].unsqueeze(2),
                        red1[:].rearrange("p (sl pt) -> p sl pt", pt=2),
                        AX.X, AL.add)
                    return out

                v_re = wsum(cwfre_sb, "tmpa", "reda", "vre")
                v_im = wsum(cwfim_sb, "tmpb", "redb", "vim")
                last = (s == S - 1)
                if not last:
                    rp_re = wsum(rwfre_sb, "tmpc", "redc", "rpre")
                    rp_im = wsum(rwfim_sb, "tmpd", "redd", "rpim")

                _sc = [0]

                def t8(a, b, op):
                    _sc[0] += 1
                    out = pcp.tile([128, 8], f32, name=f"cs{s}_{_sc[0]}",
                                   tag=f"cs{_sc[0] % 12}", bufs=2)
                    nc.vector.tensor_tensor(out[:], a, b, op)
                    return out[:]

                def cmul(wre_v, wim_v, zre, zim):
                    re = t8(t8(wre_v, zre, AL.mult), t8(wim_v, zim, AL.mult),
                            AL.subtract)
                    im = t8(t8(wre_v, zim, AL.mult), t8(wim_v, zre, AL.mult),
                            AL.add)
                    return re, im

                R_re, R_im = state["R_re"], state["R_im"]
                # V_total = Vsame + Q^c * R_prev
                qr_re, qr_im = cmul(ccv(0, 0), ccv(0, 1), R_re[:], R_im[:])
                vt_re = t8(v_re[:], qr_re, AL.add)
                vt_im = t8(v_im[:], qr_im, AL.add)
                # V' = ROT1 * V_total
                vp_re, vp_im = cmul(ccv(3, 0), ccv(3, 1), vt_re, vt_im)
                if not last:
                    # R_new = Q8*R + Rpart
                    q8r_re, q8r_im = cmul(ccv(4, 0), ccv(4, 1), R_re[:],
                                          R_im[:])
                    rn_re = pcp.tile([128, 8], f32, name=f"rnre{s}", tag="rn",
                                     bufs=2)
                    rn_im = pcp.tile([128, 8], f32, name=f"rnim{s}", tag="rn2",
                                     bufs=2)
                    nc.vector.tensor_tensor(rn_re[:], q8r_re, rp_re[:], AL.add)
                    nc.vector.tensor_tensor(rn_im[:], q8r_im, rp_im[:], AL.add)
                    state["R_re"], state["R_im"] = rn_re, rn_im

                # u = u1 + Pc*v_re + Ps*v_im ; the two scaled-table products
                # run on the (idle) Scalar engine, the adds on DVE.
                for sl in range(NSL):
                    d1 = pg.tile([128, T], f16, name=f"d1_{s}_{sl}", tag="d1",
                                 bufs=3)
                    nc.scalar.mul(d1[:], pcs_sb[sl][:, 0:T],
                                  vp_re[:, sl:sl + 1])
                    d2 = pg.tile([128, T], f16, name=f"d2_{s}_{sl}", tag="d2",
                                 bufs=3)
                    nc.scalar.mul(d2[:], pcs_sb[sl][:, T:2 * T],
                                  vp_im[:, sl:sl + 1])
                    nc.vector.tensor_tensor(u1_t[sl][:], u1_t[sl][:], d1[:],
                                            AL.add)
                    nc.vector.tensor_tensor(u1_t[sl][:], u1_t[sl][:], d2[:],
                                            AL.add)

            def emit_consume_y(s):
                sv = saved.pop(s)
                xt_sb = sv["xt_sb"]
                u1_t = sv["u1_t"]
                for n in range(NSL):
                    ns = slice(n * 128, (n + 1) * 128)
                    psy = pp.tile([128, T], f32, name=f"psy{s}_{n}", tag="ytile",
                                  bufs=3)
                    # D*x first: it needs no carry-folded u, so it can fill
                    # the PE while the fold wave finishes.
                    nc.tensor.matmul(psy[:], ddg_sb[:, ns], xt_sb[n][:],
                                     start=True, stop=False)
                    for sl in range(NSL):
                        nc.tensor.matmul(psy[:], ct_sb[sl][:, ns], u1_t[sl][:],
                                         start=False, stop=(sl == NSL - 1))
                    ye = pg.tile([128, T], f16, name=f"ye{s}_{n}", tag="ye",
                                 bufs=2)
                    nc.scalar.copy(ye[:], psy[:])
                    nc.sync.dma_start(y_d[s, ns, :], ye[:])

            emit_xt(0)
            emit_slot(0, after_xt=emit_deferred_tables)
            emit_publish(0)
            emit_xt(1)
            emit_slot(1)
            emit_publish(1)
            emit_xt(2)
            emit_consume_carry(0)
            emit_slot(2)
            emit_consume_y(0)
            emit_publish(2)
            emit_xt(3)
            emit_consume_carry(1)
            emit_slot(3)
            emit_consume_y(1)
            emit_publish(3)
            emit_consume_carry(2)
            emit_consume_carry(3)
            emit_consume_y(2)
            emit_consume_y(3)

    nc.compile()
    _BUILD_CACHE["nc"] = nc
    return nc


def _prep(inputs, A_re, A_im, B_re, B_im, C, D):
    x = np.asarray(inputs, dtype=np.float32)
    A_re = np.asarray(A_re, dtype=np.float32)
    A_im = np.asarray(A_im, dtype=np.float32)
    B_re = np.asarray(B_re, dtype=np.float32)
    B_im = np.asarray(B_im, dtype=np.float32)
    C = np.asarray(C, dtype=np.float32)
    D = np.asarray(D, dtype=np.float32)
    A = A_re.astype(np.float64) + 1j * A_im.astype(np.float64)
    r = np.abs(A)
    th = np.angle(A)
    k = np.arange(T)
    COS = np.cos(th[:, None] * k).astype(np.float32)
    SIN = np.sin(th[:, None] * k).astype(np.float32)
    RPOW = (r[:, None] ** (k + 1)).astype(np.float32)
    Q = A ** T
    ROT1 = np.exp(1j * th)
    ROTT1 = np.exp(1j * th * (T - 1))
    Q8 = Q ** 8
    RW = [Q ** (7 - j) for j in range(NC)]

    brt = np.ascontiguousarray(B_re.T).astype(np.float16)
    bit = np.ascontiguousarray(B_im.T).astype(np.float16)
    ct = np.ascontiguousarray(C.T).astype(np.float16)
    trig = np.concatenate([COS, -SIN, SIN], axis=1).astype(np.float16)
    pcs = np.concatenate([COS * RPOW, -SIN * RPOW], axis=1).astype(np.float16)
    ident = np.eye(128, dtype=np.float16)
    ddg = np.zeros((128, M), np.float16)
    for n in range(NSL):
        ddg[:, n * 128:(n + 1) * 128] = np.diag(D[n * 128:(n + 1) * 128])

    xT = np.ascontiguousarray(x.T)  # [M, L]

    def cvec_rows(z):
        # complex [H] -> rows [16, 128] (row = 2*sl + pt)
        out = np.zeros((16, 128), np.float32)
        zr = z.real.astype(np.float32).reshape(8, 128)
        zi = z.imag.astype(np.float32).reshape(8, 128)
        out[0::2] = zr
        out[1::2] = zi
        return out

    rwf_re = np.zeros((128, 128), np.float32)
    rwf_im = np.zeros((128, 128), np.float32)
    for j in range(NC):
        w = RW[j]
        wr = w.real.astype(np.float32).reshape(8, 128)
        wi = w.imag.astype(np.float32).reshape(8, 128)
        for sl in range(8):
            rwf_re[:, 16 * j + 2 * sl + 0] = wr[sl]
            rwf_re[:, 16 * j + 2 * sl + 1] = -wi[sl]
            rwf_im[:, 16 * j + 2 * sl + 0] = wi[sl]
            rwf_im[:, 16 * j + 2 * sl + 1] = wr[sl]

    in_maps = []
    for c in range(NC):
        QPC = Q ** c
        consts = np.zeros((128, 128), np.float32)
        consts[0:16] = cvec_rows(QPC)
        consts[16:32] = cvec_rows(Q ** (c + 1))
        consts[32:48] = cvec_rows(ROTT1)
        consts[48:64] = cvec_rows(ROT1)
        consts[64:80] = cvec_rows(Q8)
        # block 5: row 80+2*sl = D slice, row 81+2*sl = r slice
        consts[80:96] = cvec_rows(D.astype(np.float64) + 1j * r)
        cTm = np.ascontiguousarray(consts.T)

        cwf_re = np.zeros((128, 128), np.float32)
        cwf_im = np.zeros((128, 128), np.float32)
        for j in range(c):
            w = Q ** (c - 1 - j)
            wr = w.real.astype(np.float32).reshape(8, 128)
            wi = w.imag.astype(np.float32).reshape(8, 128)
            for sl in range(8):
                cwf_re[:, 16 * j + 2 * sl + 0] = wr[sl]
                cwf_re[:, 16 * j + 2 * sl + 1] = -wi[sl]
                cwf_im[:, 16 * j + 2 * sl + 0] = wi[sl]
                cwf_im[:, 16 * j + 2 * sl + 1] = wr[sl]

        xt = np.zeros((S, M, T), np.float16)
        for s in range(S):
            m = 8 * s + c
            xt[s] = xT[:, m * T:(m + 1) * T]

        in_maps.append({
            "xt": xt, "brt": brt, "bit": bit, "ct": ct,
            "trig": trig, "pcs": pcs,
            "cT": cTm,
            "cwfre": cwf_re, "cwfim": cwf_im,
            "rwfre": rwf_re, "rwfim": rwf_im,
            "ident": ident, "ddg": ddg,
        })
    return in_maps


LAST_RESULTS = {}


def kernel(inputs, A_re, A_im, B_re, B_im, C, D):
    nc = _build()
    in_maps = _prep(inputs, A_re, A_im, B_re, B_im, C, D)
    trace = bool(os.environ.get("DIAG_TRACE"))
    res = run_bass_kernel_spmd(nc, in_maps, core_ids=list(range(NC)),
                               trace=trace)
    LAST_RESULTS["exec_time_ns"] = res.exec_time_ns
    LAST_RESULTS["mean_exec_time_ns"] = res.mean_exec_time_ns
    yT = np.zeros((M, L), np.float32)
    for c in range(NC):
        yc = res.results[c]["y"].astype(np.float32)
        for s in range(S):
            m = 8 * s + c
            yT[:, m * T:(m + 1) * T] = yc[s]
    return np.ascontiguousarray(yT.T)


# revision 36
# speedup vs baseline: 1.1710x; 1.1710x over previous
"""Trainium2 Bass kernel for nn_DiagRNN (diagonal complex linear RNN / LRU).

  y = Re[C @ h] + D*x,  h_t = A h_{t-1} + B x_t  (A complex-diagonal)

Strategy (8 NeuronCores, sequence-parallel), v5:
  * L=16384 split into 32 chunks of T=512; chunk m on core m%8, slot m//8.
    Slots processed sequentially (4 phases) so the per-slot AllGather
    barriers keep cores loosely synced; consume work for slot s is emitted
    behind slot s+2's B matmuls (engine queues are in-order).
  * Complex scan -> two real scans per chunk via rotating-frame transform.
    Elementwise rotation works on packed [re|im] tiles [128,1024]:
        P  = [p_re | p_im]             (Act copies from PSUM)
        CP = [cos|cos] * P             (DVE, broadcast-pair view)
        SP = [-sin|+sin] * P           (DVE)
        g  = [CP_L + SP_R | CP_R + SP_L] = [g_re | g_im]  (2 DVE adds)
        W  = scan(r, g) per half       (DVE hw scan, zero-init, fp32 r)
        CW = [cos|-sin] * W            (DVE)
        u1 = CW_L + CW_R               (DVE)  -- carry-free output
    One [H, 3T] table [cos | -sin | +sin] serves CP/SP/CW.
  * Carries: chunk sums E (fp16) AllGather'd per slot; predecessor folds
    via small DVE ops; carry enters u via fused affine_then_add with
    tables Pc=cos*rpow, Ps=-sin*rpow.
  * y = C@u + diag(D)@x fused into the PSUM accumulation of the C matmul.
  * An early dummy AllGather absorbs cross-core launch skew.
"""
import sys, os
sys.path.insert(0, '/opt/trn_rl_repo')
import numpy as np

import concourse.bass as bass
import concourse.bacc as bacc
import concourse.tile as tile
import concourse.mybir as mybir
from concourse.bass_utils import run_bass_kernel_spmd

L, H, M = 16384, 1024, 1024
NC = 8
T = 512
S = L // (T * NC)          # 4 slots
NSL = H // 128             # 8 slices

f32 = mybir.dt.float32
f16 = mybir.dt.float16
AL = mybir.AluOpType
AX = mybir.AxisListType

_BUILD_CACHE = {}


def _build():
    if "nc" in _BUILD_CACHE:
        return _BUILD_CACHE["nc"]
    nc = bacc.Bacc("TRN2", target_bir_lowering=False, debug=False,
                   num_devices=NC)

    xt_d = nc.dram_tensor("xt", [S, M, T], f16, kind="ExternalInput").ap()
    brt_d = nc.dram_tensor("brt", [M, H], f16, kind="ExternalInput").ap()
    bit_d = nc.dram_tensor("bit", [M, H], f16, kind="ExternalInput").ap()
    ct_d = nc.dram_tensor("ct", [H, M], f16, kind="ExternalInput").ap()
    trig_d = nc.dram_tensor("trig", [H, 3 * T], f16, kind="ExternalInput").ap()
    pcs_d = nc.dram_tensor("pcs", [H, 2 * T], f16, kind="ExternalInput").ap()
    cT_d = nc.dram_tensor("cT", [128, 128], f32, kind="ExternalInput").ap()
    cwfre_d = nc.dram_tensor("cwfre", [128, 128], f32, kind="ExternalInput").ap()
    cwfim_d = nc.dram_tensor("cwfim", [128, 128], f32, kind="ExternalInput").ap()
    rwfre_d = nc.dram_tensor("rwfre", [128, 128], f32, kind="ExternalInput").ap()
    rwfim_d = nc.dram_tensor("rwfim", [128, 128], f32, kind="ExternalInput").ap()
    ident_d = nc.dram_tensor("ident", [128, 128], f16, kind="ExternalInput").ap()
    ddg_d = nc.dram_tensor("ddg", [128, M], f16, kind="ExternalInput").ap()
    y_d = nc.dram_tensor("y", [S, M, T], f16, kind="ExternalOutput").ap()

    with tile.TileContext(nc) as tc:
        with tc.tile_pool(name="pw", bufs=1) as pw, \
             tc.tile_pool(name="px", bufs=1) as px, \
             tc.tile_pool(name="pg", bufs=1) as pg, \
             tc.tile_pool(name="pc", bufs=1) as pcp, \
             tc.tile_pool(name="pp", bufs=1, space="PSUM") as pp, \
             tc.tile_pool(name="pd", bufs=1, space="DRAM") as pd:

            # ---------- persistent weights / tables ----------
            brt_sb = []
            bit_sb = []
            ct_sb = []
            trig_sb = []
            pcs_sb = []
            for d in range(NSL):
                brt_sb.append(pw.tile([128, H], f16, name=f"brt{d}"))
                bit_sb.append(pw.tile([128, H], f16, name=f"bit{d}"))
                ct_sb.append(pw.tile([128, M], f16, name=f"ct{d}"))
                trig_sb.append(pw.tile([128, 3 * T], f16, name=f"trig{d}"))
                pcs_sb.append(pw.tile([128, 2 * T], f16, name=f"pcs{d}"))

            ident_sb = pw.tile([128, 128], f16, name="ident")
            nc.sync.dma_start(ident_sb[:], ident_d)
            ddg_sb = pw.tile([128, M], f16, name="ddg")
            nc.sync.dma_start(ddg_sb[:], ddg_d)
            cT = pw.tile([128, 128], f32, name="cT")
            nc.sync.dma_start(cT[:], cT_d)
            cwfre_sb = pw.tile([128, 128], f32, name="cwfre")
            cwfim_sb = pw.tile([128, 128], f32, name="cwfim")
            rwfre_sb = pw.tile([128, 128], f32, name="rwfre")
            rwfim_sb = pw.tile([128, 128], f32, name="rwfim")

            # early sync: tiny AllGather to absorb cross-core launch skew
            dum_dr = pd.tile([16, 128], f16, name="dumdr")
            nc.sync.dma_start(dum_dr[:], ident_sb[0:16, :])

            dumg_dr = pd.tile([128, 128], f16, name="dumg", addr_space="Shared")
            nc.gpsimd.collective_compute(
                "AllGather", AL.bypass,
                replica_groups=[list(range(NC))],
                ins=[dum_dr[:].opt()],
                outs=[dumg_dr[:].opt()],
            )

            def emit_deferred_tables():
                for d in range(2):
                    nc.sync.dma_start(trig_sb[d][:], trig_d[d * 128:(d + 1) * 128, :])
                for d in range(NSL):
                    nc.sync.dma_start(bit_sb[d][:], bit_d[d * 128:(d + 1) * 128, :])
                for d in range(2, NSL):
                    nc.sync.dma_start(trig_sb[d][:], trig_d[d * 128:(d + 1) * 128, :])
                for d in range(NSL):
                    nc.sync.dma_start(ct_sb[d][:], ct_d[d * 128:(d + 1) * 128, :])
                    nc.sync.dma_start(pcs_sb[d][:], pcs_d[d * 128:(d + 1) * 128, :])
                nc.sync.dma_start(cwfre_sb[:], cwfre_d)
                nc.sync.dma_start(cwfim_sb[:], cwfim_d)
                nc.sync.dma_start(rwfre_sb[:], rwfre_d)
                nc.sync.dma_start(rwfim_sb[:], rwfim_d)

            def ccv(q, pt):
                # [128, 8] strided view of const block q, part pt (0=re,1=im)
                return cT[:, 16 * q + pt:16 * q + 16:2]

            def ccol(q, sl, pt):
                return cT[:, 16 * q + 2 * sl + pt:16 * q + 2 * sl + pt + 1]

            # persistent carry state
            zR_re = pcp.tile([128, 8], f32, name="zR_re")
            zR_im = pcp.tile([128, 8], f32, name="zR_im")
            nc.vector.memzero(zR_re[:])
            nc.vector.memzero(zR_im[:])
            state = {"R_re": zR_re, "R_im": zR_im}
            saved = {}
            xt_tiles = {}

            def emit_xt(s):
                xt_sb = []
                for d in range(NSL):
                    t_ = px.tile([128, T], f16, name=f"xt_s{s}_d{d}",
                                 tag="xt", bufs=24)
                    nc.sync.dma_start(t_[:], xt_d[s, d * 128:(d + 1) * 128, :])
                    xt_sb.append(t_)
                    if s == 0:
                        nc.sync.dma_start(brt_sb[d][:],
                                          brt_d[d * 128:(d + 1) * 128, :])
                xt_tiles[s] = xt_sb

            def emit_slot(s, after_xt=None):
                """B matmuls + rotate + scans + E publish for slot s."""
                xt_sb = xt_tiles[s]
                if after_xt is not None:
                    after_xt()

                u1_t = []
                wlre = pcp.tile([128, 8], f16, name=f"wlre{s}", tag="wl", bufs=2)
                wlim = pcp.tile([128, 8], f16, name=f"wlim{s}", tag="wl2", bufs=2)

                for sl in range(NSL):
                    hs = slice(sl * 128, (sl + 1) * 128)
                    ps_re = pp.tile([128, T], f32, name=f"psre{s}_{sl}",
                                    tag="bu", bufs=4)
                    ps_im = pp.tile([128, T], f32, name=f"psim{s}_{sl}",
                                    tag="bu", bufs=4)
                    for d in range(NSL):
                        nc.tensor.matmul(ps_re[:], brt_sb[d][:, hs], xt_sb[d][:],
                                         start=(d == 0), stop=(d == NSL - 1))
                    for d in range(NSL):
                        nc.tensor.matmul(ps_im[:], bit_sb[d][:, hs], xt_sb[d][:],
                                         start=(d == 0), stop=(d == NSL - 1))

                    P = pg.tile([128, 2 * T], f16, name=f"P{s}_{sl}",
                                tag="P", bufs=4)
                    nc.scalar.copy(P[:, 0:T], ps_re[:])
                    nc.scalar.copy(P[:, T:2 * T], ps_im[:])
                    CP = pg.tile([128, 2 * T], f16, name=f"cp{s}_{sl}",
                                 tag="cp", bufs=3)
                    cosD = trig_sb[sl][:, 0:T].unsqueeze(1).broadcast_to(
                        [128, 2, T])
                    nc.vector.tensor_tensor(
                        CP[:].rearrange("a (b c) -> a b c", b=2),
                        cosD, P[:].rearrange("a (b c) -> a b c", b=2),
                        AL.mult)
                    SP = pg.tile([128, 2 * T], f16, name=f"sp{s}_{sl}",
                                 tag="sp", bufs=3)
                    nc.vector.tensor_tensor(SP[:], trig_sb[sl][:, T:3 * T],
                                            P[:], AL.mult)
                    g = pg.tile([128, 2 * T], f16, name=f"g{s}_{sl}",
                                tag="g", bufs=3)
                    nc.vector.tensor_tensor(g[:, 0:T], CP[:, 0:T],
                                            SP[:, T:2 * T], AL.add)
                    nc.vector.tensor_tensor(g[:, T:2 * T], CP[:, T:2 * T],
                                            SP[:, 0:T], AL.add)
                    W = pg.tile([128, 2 * T], f16, name=f"w{s}_{sl}",
                                tag="w", bufs=4)
                    rdec = ccol(5, sl, 1).broadcast_to([128, T])
                    nc.vector.tensor_tensor_scan(W[:, 0:T], rdec, g[:, 0:T],
                                                 0.0, AL.mult, AL.add)
                    nc.vector.tensor_tensor_scan(W[:, T:2 * T], rdec,
                                                 g[:, T:2 * T],
                                                 0.0, AL.mult, AL.add)
                    CW = pg.tile([128, 2 * T], f16, name=f"cw{s}_{sl}",
                                 tag="cw", bufs=3)
                    nc.vector.tensor_tensor(CW[:], trig_sb[sl][:, 0:2 * T],
                                            W[:], AL.mult)
                    u1 = pg.tile([128, T], f16, name=f"u1_{s}_{sl}",
                                 tag="u1", bufs=18)
                    nc.vector.tensor_tensor(u1[:], CW[:, 0:T],
                                            CW[:, T:2 * T], AL.add)
                    u1_t.append(u1)
                    nc.scalar.copy(wlre[:, sl:sl + 1], W[:, T - 1:T])
                    nc.scalar.copy(wlim[:, sl:sl + 1], W[:, 2 * T - 1:2 * T])

                saved[s] = dict(xt_sb=xt_sb, u1_t=u1_t, wlre=wlre, wlim=wlim)

            def emit_publish(s):
                sv = saved[s]
                wlre, wlim = sv["wlre"], sv["wlim"]
                # E publish: E = ROTT1 * W_last, fp16
                epack = pcp.tile([128, 16], f16, name=f"epack{s}", tag="ep",
                                 bufs=2)
                sa = pcp.tile([128, 8], f32, name=f"sa{s}", tag="sa", bufs=2)
                sb_ = pcp.tile([128, 8], f32, name=f"sb{s}", tag="sb", bufs=2)
                sc_ = pcp.tile([128, 8], f32, name=f"sc{s}", tag="sc", bufs=2)
                sd = pcp.tile([128, 8], f32, name=f"sd{s}", tag="sd", bufs=2)
                nc.vector.tensor_tensor(sa[:], ccv(2, 0), wlre[:], AL.mult)
                nc.vector.tensor_tensor(sb_[:], ccv(2, 1), wlim[:], AL.mult)
                nc.vector.tensor_tensor(epack[:, 0:16:2], sa[:], sb_[:],
                                        AL.subtract)
                nc.vector.tensor_tensor(sc_[:], ccv(2, 0), wlim[:], AL.mult)
                nc.vector.tensor_tensor(sd[:], ccv(2, 1), wlre[:], AL.mult)
                nc.vector.tensor_tensor(epack[:, 1:16:2], sc_[:], sd[:],
                                        AL.add)

                pub_ps = pp.tile([16, 128], f16, name=f"pubps{s}", tag="tp",
                                 bufs=1)
                nc.tensor.transpose(pub_ps[:], epack[:], ident_sb[:])
                pub_sb = pcp.tile([16, 128], f16, name=f"pubsb{s}",
                                  tag="pub", bufs=2)
                nc.vector.tensor_copy(pub_sb[:], pub_ps[:])
                pub_dr = pd.tile([16, 128], f16, name=f"pubdr{s}",
                                 tag="pubd", bufs=2)
                nc.sync.dma_start(pub_dr[:], pub_sb[:])
                gat_dr = pd.tile([128, 128], f16, name=f"gatdr{s}",
                                 tag="gatd", bufs=2, addr_space="Shared")
                nc.gpsimd.collective_compute(
                    "AllGather", AL.bypass,
                    replica_groups=[list(range(NC))],
                    ins=[pub_dr[:].opt()],
                    outs=[gat_dr[:].opt()],
                )
                sv["gat_dr"] = gat_dr

            def emit_consume_carry(s):
                sv = saved[s]
                u1_t = sv["u1_t"]
                eg = pcp.tile([128, 128], f16, name=f"eg{s}", tag="eg", bufs=2)
                nc.sync.dma_start(eg[:], sv["gat_dr"][:])
                et = pcp.tile([128, 128], f16, name=f"et{s}", tag="et", bufs=2)
                nc.sync.dma_start_transpose(et[:], eg[:])

                def wsum(fold_sb, nmv, nmr1, nm):
                    tmp = pcp.tile([128, 128], f32, name=f"{nmv}{s}", tag="redt",
                                   bufs=2)
                    nc.vector.tensor_tensor(tmp[:], fold_sb[:], et[:], AL.mult)
                    red1 = pcp.tile([128, 16], f32, name=f"{nmr1}{s}",
                                    tag="red1", bufs=2)
                    nc.vector.tensor_reduce(
                        red1[:].unsqueeze(2),
                        tmp[:].rearrange("p (j x) -> p x j", j=8),
                        AX.X, AL.add)
                    out = pcp.tile([128, 8], f32, name=f"{nm}{s}", tag=nm,
                                   bufs=2)
                    nc.vector.tensor_reduce(
                        out[:].unsqueeze(2),
                        red1[:].rearrange("p (sl pt) -> p sl pt", pt=2),
                        AX.X, AL.add)
                    return out

                v_re = wsum(cwfre_sb, "tmpa", "reda", "vre")
                v_im = wsum(cwfim_sb, "tmpb", "redb", "vim")
                last = (s == S - 1)
                if not last:
                    rp_re = wsum(rwfre_sb, "tmpc", "redc", "rpre")
                    rp_im = wsum(rwfim_sb, "tmpd", "redd", "rpim")

                _sc = [0]

                def t8(a, b, op):
                    _sc[0] += 1
                    out = pcp.tile([128, 8], f32, name=f"cs{s}_{_sc[0]}",
                                   tag=f"cs{_sc[0] % 12}", bufs=2)
                    nc.vector.tensor_tensor(out[:], a, b, op)
                    return out[:]

                def cmul(wre_v, wim_v, zre, zim):
                    re = t8(t8(wre_v, zre, AL.mult), t8(wim_v, zim, AL.mult),
                            AL.subtract)
                    im = t8(t8(wre_v, zim, AL.mult), t8(wim_v, zre, AL.mult),
                            AL.add)
                    return re, im

                R_re, R_im = state["R_re"], state["R_im"]
                # V_total = Vsame + Q^c * R_prev
                qr_re, qr_im = cmul(ccv(0, 0), ccv(0, 1), R_re[:], R_im[:])
                vt_re = t8(v_re[:], qr_re, AL.add)
                vt_im = t8(v_im[:], qr_im, AL.add)
                # V' = ROT1 * V_total
                vp_re, vp_im = cmul(ccv(3, 0), ccv(3, 1), vt_re, vt_im)
                if not last:
                    # R_new = Q8*R + Rpart
                    q8r_re, q8r_im = cmul(ccv(4, 0), ccv(4, 1), R_re[:],
                                          R_im[:])
                    rn_re = pcp.tile([128, 8], f32, name=f"rnre{s}", tag="rn",
                                     bufs=2)
                    rn_im = pcp.tile([128, 8], f32, name=f"rnim{s}", tag="rn2",
                                     bufs=2)
                    nc.vector.tensor_tensor(rn_re[:], q8r_re, rp_re[:], AL.add)
                    nc.vector.tensor_tensor(rn_im[:], q8r_im, rp_im[:], AL.add)
                    state["R_re"], state["R_im"] = rn_re, rn_im

                # u = u1 + Pc*v_re + Ps*v_im ; the two scaled-table products
                # run on the (idle) Scalar engine, the adds on DVE.
                for sl in range(NSL):
                    d1 = pg.tile([128, T], f16, name=f"d1_{s}_{sl}", tag="d1",
                                 bufs=3)
                    nc.scalar.mul(d1[:], pcs_sb[sl][:, 0:T],
                                  vp_re[:, sl:sl + 1])
                    d2 = pg.tile([128, T], f16, name=f"d2_{s}_{sl}", tag="d2",
                                 bufs=3)
                    nc.scalar.mul(d2[:], pcs_sb[sl][:, T:2 * T],
                                  vp_im[:, sl:sl + 1])
                    nc.vector.tensor_tensor(u1_t[sl][:], u1_t[sl][:], d1[:],
                                            AL.add)
                    nc.vector.tensor_tensor(u1_t[sl][:], u1_t[sl][:], d2[:],
                                            AL.add)

            def emit_consume_y(s):
                sv = saved.pop(s)
                xt_sb = sv["xt_sb"]
                u1_t = sv["u1_t"]
                for n in range(NSL):
                    ns = slice(n * 128, (n + 1) * 128)
                    psy = pp.tile([128, T], f32, name=f"psy{s}_{n}", tag="ytile",
                                  bufs=3)
                    # D*x first: it needs no carry-folded u, so it can fill
                    # the PE while the fold wave finishes.
                    nc.tensor.matmul(psy[:], ddg_sb[:, ns], xt_sb[n][:],
                                     start=True, stop=False)
                    for sl in range(NSL):
                        nc.tensor.matmul(psy[:], ct_sb[sl][:, ns], u1_t[sl][:],
                                         start=False, stop=(sl == NSL - 1))
                    ye = pg.tile([128, T], f16, name=f"ye{s}_{n}", tag="ye",
                                 bufs=2)
                    nc.scalar.copy(ye[:], psy[:])
                    nc.sync.dma_start(y_d[s, ns, :], ye[:])

            emit_xt(0)
            emit_slot(0, after_xt=emit_deferred_tables)
            emit_publish(0)
            emit_xt(1)
            emit_slot(1)
            emit_publish(1)
            emit_xt(2)
            emit_consume_carry(0)
            emit_slot(2)
            emit_consume_y(0)
            emit_publish(2)
            emit_xt(3)
            emit_consume_carry(1)
            emit_slot(3)
            emit_consume_y(1)
            emit_publish(3)
            emit_consume_carry(2)
            emit_consume_y(2)
            emit_consume_carry(3)
            emit_consume_y(3)

    nc.compile()
    _BUILD_CACHE["nc"] = nc
    return nc


def _prep(inputs, A_re, A_im, B_re, B_im, C, D):
    x = np.asarray(inputs, dtype=np.float32)
    A_re = np.asarray(A_re, dtype=np.float32)
    A_im = np.asarray(A_im, dtype=np.float32)
    B_re = np.asarray(B_re, dtype=np.float32)
    B_im = np.asarray(B_im, dtype=np.float32)
    C = np.asarray(C, dtype=np.float32)
    D = np.asarray(D, dtype=np.float32)
    A = A_re.astype(np.float64) + 1j * A_im.astype(np.float64)
    r = np.abs(A)
    th = np.angle(A)
    k = np.arange(T)
    COS = np.cos(th[:, None] * k).astype(np.float32)
    SIN = np.sin(th[:, None] * k).astype(np.float32)
    RPOW = (r[:, None] ** (k + 1)).astype(np.float32)
    Q = A ** T
    ROT1 = np.exp(1j * th)
    ROTT1 = np.exp(1j * th * (T - 1))
    Q8 = Q ** 8
    RW = [Q ** (7 - j) for j in range(NC)]

    brt = np.ascontiguousarray(B_re.T).astype(np.float16)
    bit = np.ascontiguousarray(B_im.T).astype(np.float16)
    ct = np.ascontiguousarray(C.T).astype(np.float16)
    trig = np.concatenate([COS, -SIN, SIN], axis=1).astype(np.float16)
    pcs = np.concatenate([COS * RPOW, -SIN * RPOW], axis=1).astype(np.float16)
    ident = np.eye(128, dtype=np.float16)
    ddg = np.zeros((128, M), np.float16)
    for n in range(NSL):
        ddg[:, n * 128:(n + 1) * 128] = np.diag(D[n * 128:(n + 1) * 128])

    xT = np.ascontiguousarray(x.T)  # [M, L]

    def cvec_rows(z):
        # complex [H] -> rows [16, 128] (row = 2*sl + pt)
        out = np.zeros((16, 128), np.float32)
        zr = z.real.astype(np.float32).reshape(8, 128)
        zi = z.imag.astype(np.float32).reshape(8, 128)
        out[0::2] = zr
        out[1::2] = zi
        return out

    rwf_re = np.zeros((128, 128), np.float32)
    rwf_im = np.zeros((128, 128), np.float32)
    for j in range(NC):
        w = RW[j]
        wr = w.real.astype(np.float32).reshape(8, 128)
        wi = w.imag.astype(np.float32).reshape(8, 128)
        for sl in range(8):
            rwf_re[:, 16 * j + 2 * sl + 0] = wr[sl]
            rwf_re[:, 16 * j + 2 * sl + 1] = -wi[sl]
            rwf_im[:, 16 * j + 2 * sl + 0] = wi[sl]
            rwf_im[:, 16 * j + 2 * sl + 1] = wr[sl]

    in_maps = []
    for c in range(NC):
        QPC = Q ** c
        consts = np.zeros((128, 128), np.float32)
        consts[0:16] = cvec_rows(QPC)
        consts[16:32] = cvec_rows(Q ** (c + 1))
        consts[32:48] = cvec_rows(ROTT1)
        consts[48:64] = cvec_rows(ROT1)
        consts[64:80] = cvec_rows(Q8)
        # block 5: row 80+2*sl = D slice, row 81+2*sl = r slice
        consts[80:96] = cvec_rows(D.astype(np.float64) + 1j * r)
        cTm = np.ascontiguousarray(consts.T)

        cwf_re = np.zeros((128, 128), np.float32)
        cwf_im = np.zeros((128, 128), np.float32)
        for j in range(c):
            w = Q ** (c - 1 - j)
            wr = w.real.astype(np.float32).reshape(8, 128)
            wi = w.imag.astype(np.float32).reshape(8, 128)
            for sl in range(8):
                cwf_re[:, 16 * j + 2 * sl + 0] = wr[sl]
                cwf_re[:, 16 * j + 2 * sl + 1] = -wi[sl]
                cwf_im[:, 16 * j + 2 * sl + 0] = wi[sl]
                cwf_im[:, 16 * j + 2 * sl + 1] = wr[sl]

        xt = np.zeros((S, M, T), np.float16)
        for s in range(S):
            m = 8 * s + c
            xt[s] = xT[:, m * T:(m + 1) * T]

        in_maps.append({
            "xt": xt, "brt": brt, "bit": bit, "ct": ct,
            "trig": trig, "pcs": pcs,
            "cT": cTm,
            "cwfre": cwf_re, "cwfim": cwf_im,
            "rwfre": rwf_re, "rwfim": rwf_im,
            "ident": ident, "ddg": ddg,
        })
    return in_maps


LAST_RESULTS = {}


def kernel(inputs, A_re, A_im, B_re, B_im, C, D):
    nc = _build()
    in_maps = _prep(inputs, A_re, A_im, B_re, B_im, C, D)
    trace = bool(os.environ.get("DIAG_TRACE"))
    res = run_bass_kernel_spmd(nc, in_maps, core_ids=list(range(NC)),
                               trace=trace)
    LAST_RESULTS["exec_time_ns"] = res.exec_time_ns
    LAST_RESULTS["mean_exec_time_ns"] = res.mean_exec_time_ns
    yT = np.zeros((M, L), np.float32)
    for c in range(NC):
        yc = res.results[c]["y"].astype(np.float32)
        for s in range(S):
            m = 8 * s + c
            yT[:, m * T:(m + 1) * T] = yc[s]
    return np.ascontiguousarray(yT.T)
